# revision 10
# baseline (speedup 1.0000x reference)
"""Trainium2 Bass kernel for nn_EquivariantProductBasisBlock.

Computation (per node n, channel c):
  s = nf[n,c,0]; v = nf[n,c,1:4]; v2 = |v|^2
  out0 = w0*s + w1*s^2 + w2'*v2 + w3*s^3 + w4*s*v2      (w_p = W0[sp[n],p,c])
  B1   = u0 + u1'*s + u2'*s^2 + u3'*v2                  (u_p = W1[sp[n],p,c])
  out1m = B1 * v_m
  y0 = out0 @ L0 / sqrt(C);  y1m = out1m @ L1 / sqrt(C)
  y[n,c,:] = [y0, y1x, y1y, y1z] + sc[n,c,:]

Design (vs the node-major baseline):
  - Host sorts nodes by species and pads each species segment to a
    512-node block, so every device block is single-species.  The
    per-element path weights then become per-(block, channel) constants,
    delivered as a tiny [128, blocks*9] f32 table: the gather matmuls,
    their PSUM evacuation, and all PE transposes disappear.
  - Channels live on the partition axis ("cT layout"); nodes stream on
    the free axis.  Host pre-transposes nf/sc to bf16 component planes
    [c, 4, n], so the channel-mixing matmuls need no transposes at all:
    y0 = matmul(lhsT=L0, rhs=out0), y1m = matmul(lhsT=L1, rhs=out1m).
  - Weighted poly terms use dual-op tensor_scalar (x*w + b in one DVE op
    at 4x bf16 rate) with per-partition scalar APs from the table.
  - All IO is bf16 (host pre-casts): 24 B/node HBM traffic vs 48 f32.
  - sc is added on the idle TensorE: an identity matmul accumulates the
    sc planes into the same PSUM banks as the channel-mix matmuls (the
    SWDGE accumulate-DMA CCE path wedges this hardware for runs >2048
    elements, and engines have no spare throughput for a tensor add).
  - Output stored bf16, upcast on host.
  - Work is spread across engines: ACT does vx^2/vy^2 and the PSUM->SBUF
    evacuation, GPSIMD does two of the three B1*v planes, DVE does the
    rest of the elementwise chain, TensorE does channel mixing + sc.

Sharding: data-parallel over sorted node blocks across 8 cores
(8704 nodes/core incl. ~6% species padding).

Measured (interleaved reps=1 vs reps=24 A/B, this container): per-pass HW
time ~29-57 us across sessions (RPC-noise limited); the baseline kernel
(node-major f32, gather matmuls + PE transposes) measured 101968 ns with
the same methodology family.  Correctness: rel err 5.5e-3 (gate 2e-2),
dominated by bf16 input/output rounding.
"""

import numpy as np

N_CORES = 8
N_NODES = 65536
C = 128
E = 10
BLK = 512                      # single-species device block (1 PSUM bank / plane)
BLKS_PER_CHUNK = 4             # 2048-node DMA chunks (2 MiB bf16)

INV_SQ3 = 1.0 / np.sqrt(3.0)
SQ2 = float(np.sqrt(2.0))
SQ3 = float(np.sqrt(3.0))
SQ35 = float(np.sqrt(3.0 / 5.0))

_CACHE = {}


# ---------------------------------------------------------------------------
# Workarounds for the walrus build in this container: it rejects any
# instruction carrying more than one sync-wait ("Too many sync wait
# commands").  Split extra waits onto same-engine NOPs preceding the
# instruction (identical semantics: the engine queue is FIFO).
# ---------------------------------------------------------------------------
def _apply_patches():
    import concourse.tile as tile
    from concourse import mybir
    from concourse.vector_clock import ScopedClock

    if getattr(tile.TileContext, "_singlewait_patched", False):
        return

    def _patched_drain_and_barrier(self, tick_clock, wait_clock):
        nc = self.nc
        probe = nc.sync.nop()
        wait_clock.add_sem_waits(probe.ins, ScopedClock({None: tick_clock.global_clock}))
        si = probe.ins.sync_info
        waits = list(si.on_wait) if si and si.on_wait else []
        if len(waits) > 1:
            probe.ins.sync_info = type(si)(on_wait=waits[:1], on_update=[])
            for w in waits[1:]:
                extra = nc.sync.nop()
                extra.ins.sync_info = type(si)(on_wait=[w], on_update=[])
        nc.sync.drain()
        nc.all_engine_barrier()
        assert self.sems is not None
        popped = nc._tile_sem_poison_stack.pop()
        assert popped is self._sem_poison
        nc.clear_and_free_semaphores(list(self.sems.allocated().values()))
        nc.all_engine_barrier()

    _orig_commit = tile.TileContext._commit_instruction

    def _split_commit(self, inst, lazy_reg_writes=True):
        si = getattr(inst, "sync_info", None)
        if (si is not None and si.on_wait and len(si.on_wait) > 1
                and getattr(inst, "engine", mybir.EngineType.Unassigned)
                != mybir.EngineType.Unassigned):
            waits = list(si.on_wait)
            for w in waits[:-1]:
                nop = mybir.InstNoOp(name=self.nc.get_next_instruction_name(),
                                     ins=[], outs=[], engine=inst.engine)
                nop.sync_info = mybir.SyncInfo(on_wait=[w], on_update=[])
                _orig_commit(self, nop, lazy_reg_writes=False)
            inst.sync_info = mybir.SyncInfo(on_wait=[waits[-1]],
                                            on_update=list(si.on_update or []))
        return _orig_commit(self, inst, lazy_reg_writes)

    tile.TileContext._drain_and_barrier = _patched_drain_and_barrier
    tile.TileContext._commit_instruction = _split_commit
    tile.TileContext._singlewait_patched = True


def _chunk_sizes(bpc):
    """Chunk list (in blocks) for one core: full chunks + one tail."""
    sizes = [BLKS_PER_CHUNK] * (bpc // BLKS_PER_CHUNK)
    if bpc % BLKS_PER_CHUNK:
        sizes.append(bpc % BLKS_PER_CHUNK)
    return sizes


def _build_program(reps=1, bpc=17):
    import concourse.bass as bass
    import concourse.tile as tile
    from concourse import mybir
    from concourse.masks import make_identity
    from contextlib import ExitStack

    _apply_patches()
    F32 = mybir.dt.float32
    BF16 = mybir.dt.bfloat16
    nc = bass.Bass()

    P = bpc * BLK                      # nodes per core
    sizes = _chunk_sizes(bpc)

    nf_d = nc.declare_dram_parameter("nf", [C, 4 * P], BF16, isOutput=False)
    sc_d = nc.declare_dram_parameter("sc", [C, 4 * P], BF16, isOutput=False)
    w_d = nc.declare_dram_parameter("wt", [C, bpc * 9], F32, isOutput=False)
    l0_d = nc.declare_dram_parameter("l0", [C, C], BF16, isOutput=False)
    l1_d = nc.declare_dram_parameter("l1", [C, C], BF16, isOutput=False)
    out_d = nc.declare_dram_parameter("out", [C, 4 * P], BF16, isOutput=True)

    mult = mybir.AluOpType.mult
    add = mybir.AluOpType.add
    Square = mybir.ActivationFunctionType.Square

    with tile.TileContext(nc) as tc, ExitStack() as ctx:
        consts = ctx.enter_context(tc.tile_pool(name="consts", bufs=1))
        chunks = ctx.enter_context(tc.tile_pool(name="chunks", bufs=2))
        work = ctx.enter_context(tc.tile_pool(name="work", bufs=2))
        psY = ctx.enter_context(tc.tile_pool(name="psY", bufs=2, space="PSUM"))

        t_w = consts.tile([C, bpc * 9], F32)
        nc.sync.dma_start(out=t_w, in_=w_d[:, :])
        t_l0 = consts.tile([C, C], BF16)
        nc.sync.dma_start(out=t_l0, in_=l0_d[:, :])
        t_l1 = consts.tile([C, C], BF16)
        nc.sync.dma_start(out=t_l1, in_=l1_d[:, :])
        ident = consts.tile([C, C], BF16)
        make_identity(nc, ident)

        def ap(t, off, *dims):
            return bass.AP(tensor=t.tensor, offset=t.offset + off,
                           ap=[t.ap[0], *list(dims)])

        # iterate (rep, chunk)
        sched = []
        for _ in range(reps):
            base = 0
            for nb in sizes:
                sched.append((base, nb))
                base += nb
        for (cblk, nb) in sched:
            CH = nb * BLK
            col0 = 4 * cblk * BLK          # column offset of this chunk

            t_nf = chunks.tile([C, 4 * CH], BF16, tag="nf")
            nc.sync.dma_start(out=t_nf, in_=nf_d[:, col0:col0 + 4 * CH])
            t_sc = chunks.tile([C, 4 * CH], BF16, tag="sc")
            nc.sync.dma_start(out=t_sc, in_=sc_d[:, col0:col0 + 4 * CH])
            t_y = chunks.tile([C, 4 * CH], BF16, tag="y")

            t_vsq = work.tile([C, 2 * CH], BF16, tag="vsq")
            t_v2 = work.tile([C, CH], BF16, tag="v2")
            t_hb = work.tile([C, 2 * CH], BF16, tag="hb")    # [h|b]
            t_g1 = work.tile([C, CH], BF16, tag="g1")
            t_r = work.tile([C, CH], BF16, tag="r")
            t_X = work.tile([C, 4 * CH], BF16, tag="x")      # [out0|o1x|o1y|o1z]

            S = t_nf[:, 0:CH]
            vz = t_nf[:, 3 * CH:4 * CH]

            # |v|^2: ACT squares vx,vy; DVE squares vz and sums
            nc.scalar.activation(out=t_vsq, in_=t_nf[:, CH:3 * CH], func=Square)
            nc.vector.tensor_tensor(out=t_v2, in0=vz, in1=vz, op=mult)
            nc.vector.tensor_tensor(out=t_v2, in0=t_v2,
                                    in1=t_vsq[:, 0:CH], op=add)
            nc.vector.tensor_tensor(out=t_v2, in0=t_v2,
                                    in1=t_vsq[:, CH:2 * CH], op=add)

            def wcol(b, j):
                k = (cblk + b) * 9 + j
                return t_w[:, k:k + 1]

            # per-block weighted affines (dual-op tensor_scalar, 4x bf16)
            for b in range(nb):
                sb = S[:, b * BLK:(b + 1) * BLK]
                v2b = t_v2[:, b * BLK:(b + 1) * BLK]
                nc.vector.tensor_scalar(out=t_hb[:, b * BLK:(b + 1) * BLK],
                                        in0=sb, scalar1=wcol(b, 3),
                                        scalar2=wcol(b, 1), op0=mult, op1=add)
                nc.vector.tensor_scalar(out=t_hb[:, CH + b * BLK:CH + (b + 1) * BLK],
                                        in0=sb, scalar1=wcol(b, 7),
                                        scalar2=wcol(b, 6), op0=mult, op1=add)
                nc.vector.tensor_scalar(out=t_g1[:, b * BLK:(b + 1) * BLK],
                                        in0=sb, scalar1=wcol(b, 4),
                                        scalar2=wcol(b, 2), op0=mult, op1=add)
                nc.vector.tensor_scalar(out=t_r[:, b * BLK:(b + 1) * BLK],
                                        in0=v2b, scalar1=wcol(b, 8),
                                        scalar2=wcol(b, 5), op0=mult, op1=add)

            # [h2|b2] = [h1|b1] * s  (s broadcast over both halves)
            nc.vector.tensor_tensor(out=t_hb, in0=t_hb,
                                    in1=ap(t_nf, 0, [0, 2], [1, CH]), op=mult)

            # h3 = h2 + w0 (per block)
            for b in range(nb):
                nc.vector.tensor_scalar(out=t_hb[:, b * BLK:(b + 1) * BLK],
                                        in0=t_hb[:, b * BLK:(b + 1) * BLK],
                                        scalar1=wcol(b, 0), scalar2=None, op0=add)

            # out0 = h3*s + g1*v2
            nc.vector.tensor_tensor(out=t_X[:, 0:CH], in0=t_hb[:, 0:CH],
                                    in1=S, op=mult)
            nc.vector.tensor_tensor(out=t_g1, in0=t_g1, in1=t_v2, op=mult)
            nc.vector.tensor_tensor(out=t_X[:, 0:CH], in0=t_X[:, 0:CH],
                                    in1=t_g1, op=add)
            # B1 = b2 + r
            nc.vector.tensor_tensor(out=t_hb[:, CH:2 * CH],
                                    in0=t_hb[:, CH:2 * CH], in1=t_r, op=add)
            # out1 = B1 * v   (x,y on GpSimd, z on DVE)
            nc.gpsimd.tensor_tensor(out=t_X[:, CH:3 * CH],
                                    in0=ap(t_hb, CH, [0, 2], [1, CH]),
                                    in1=t_nf[:, CH:3 * CH], op=mult)
            nc.vector.tensor_tensor(out=t_X[:, 3 * CH:4 * CH],
                                    in0=t_hb[:, CH:2 * CH], in1=vz, op=mult)

            # channel mixing + sc (identity-matmul accumulate) + evacuation
            for b in range(nb):
                t_py = psY.tile([C, 4 * BLK], F32, tag="py")
                for m in range(4):
                    nc.tensor.matmul(t_py[:, m * BLK:(m + 1) * BLK],
                                     lhsT=(t_l0 if m == 0 else t_l1),
                                     rhs=t_X[:, m * CH + b * BLK:
                                             m * CH + (b + 1) * BLK],
                                     start=True, stop=False)
                    nc.tensor.matmul(t_py[:, m * BLK:(m + 1) * BLK],
                                     lhsT=ident,
                                     rhs=t_sc[:, m * CH + b * BLK:
                                              m * CH + (b + 1) * BLK],
                                     start=False, stop=True)
                # PSUM f32 -> y chunk bf16, interleaving planes into [4, CH]
                nc.scalar.copy(out=ap(t_y, b * BLK, [CH, 4], [1, BLK]),
                               in_=t_py[:, :])

            nc.sync.dma_start(out=out_d[:, col0:col0 + 4 * CH], in_=t_y)

    return nc


# ---------------------------------------------------------------------------
# Host-side data plan: sort nodes by species, pad each species segment to a
# 512 multiple, split into 8 equal per-core ranges of whole blocks.
# ---------------------------------------------------------------------------
def _plan(species):
    order = np.argsort(species, kind="stable")
    counts = np.bincount(species, minlength=E)
    seg_padded = ((counts + BLK - 1) // BLK) * BLK
    n_blocks_real = int(seg_padded.sum()) // BLK
    bpc = -(-n_blocks_real // N_CORES)
    P = bpc * BLK
    L = N_CORES * P
    idx = np.full(L, -1, np.int64)
    blk_sp = np.zeros(L // BLK, np.int64)
    pos = 0
    off = 0
    for e in range(E):
        c = int(counts[e])
        idx[pos:pos + c] = order[off:off + c]
        blk_sp[pos // BLK:(pos + int(seg_padded[e])) // BLK] = e
        off += c
        pos += int(seg_padded[e])
    return idx, blk_sp, bpc, P


def _prep_host(inputs):
    species = np.asarray(inputs["node_species"]).astype(np.int64)
    idx, blk_sp, bpc, P = _plan(species)
    L = N_CORES * P

    nf = np.ascontiguousarray(np.asarray(inputs["node_feats"], dtype=np.float32))
    sc = np.ascontiguousarray(np.asarray(inputs["sc"], dtype=np.float32))
    W0 = np.asarray(inputs["W0"], dtype=np.float32)
    W1 = np.asarray(inputs["W1"], dtype=np.float32)
    L0 = np.asarray(inputs["L0"], dtype=np.float32)
    L1 = np.asarray(inputs["L1"], dtype=np.float32)

    valid = idx >= 0
    nf_s = np.zeros((L, C, 4), np.float32)
    nf_s[valid] = nf[idx[valid]]
    sc_s = np.zeros((L, C, 4), np.float32)
    sc_s[valid] = sc[idx[valid]]

    # per-element path weights, CG constants folded in
    w0 = W0.copy()
    w0[:, 2, :] *= INV_SQ3
    u = W1.copy()
    u[:, 1, :] *= SQ2
    u[:, 2, :] *= SQ3
    u[:, 3, :] *= SQ35
    wall = np.concatenate([w0, u], axis=1)          # [E, 9, C]

    inv_sqrt_c = np.float32(1.0 / np.sqrt(C))
    l0 = np.ascontiguousarray(L0 * inv_sqrt_c)
    l1 = np.ascontiguousarray(L1 * inv_sqrt_c)
    return nf_s, sc_s, wall, l0, l1, idx, blk_sp, bpc, P


def _planes(arr_s, core, bpc, P, bf16):
    """[L,C,4] f32 -> [C, 4P] bf16 chunk-major component planes for one core."""
    a = arr_s[core * P:(core + 1) * P]              # [P, C, 4]
    cols = []
    pos = 0
    for nb in _chunk_sizes(bpc):
        CH = nb * BLK
        ch = a[pos:pos + CH]                        # [CH, C, 4]
        cols.append(ch.transpose(1, 2, 0).reshape(C, 4 * CH))
        pos += CH
    return np.ascontiguousarray(np.concatenate(cols, axis=1)).astype(bf16)


def _in_maps(inputs):
    import ml_dtypes
    bf16 = ml_dtypes.bfloat16
    nf_s, sc_s, wall, l0, l1, idx, blk_sp, bpc, P = _prep_host(inputs)
    _CACHE["plan"] = (idx, bpc, P)
    l0b = l0.astype(bf16)
    l1b = l1.astype(bf16)
    maps = []
    for k in range(N_CORES):
        bs = blk_sp[k * bpc:(k + 1) * bpc]
        wt = wall[bs]                               # [bpc, 9, C]
        wt = np.ascontiguousarray(wt.transpose(2, 0, 1).reshape(C, bpc * 9))
        maps.append({
            "nf": _planes(nf_s, k, bpc, P, bf16),
            "sc": _planes(sc_s, k, bpc, P, bf16),
            "wt": wt.astype(np.float32),
            "l0": l0b,
            "l1": l1b,
        })
    return maps


def _unpack(res, inputs):
    idx, bpc, P = _CACHE["plan"]
    L = N_CORES * P
    out_s = np.empty((L, C, 4), np.float32)
    for k in range(N_CORES):
        o = np.asarray(res.results[k]["out"]).astype(np.float32)   # [C, 4P]
        pos = 0
        col = 0
        for nb in _chunk_sizes(bpc):
            CH = nb * BLK
            ch = o[:, col:col + 4 * CH].reshape(C, 4, CH)
            out_s[k * P + pos:k * P + pos + CH] = ch.transpose(2, 0, 1)
            pos += CH
            col += 4 * CH
    valid = idx >= 0
    y = np.empty((N_NODES, C, 4), np.float32)
    y[idx[valid]] = out_s[valid]
    return y


def kernel(**inputs):
    from concourse.bass_utils import run_bass_kernel_spmd

    maps = _in_maps(inputs)
    idx, bpc, P = _CACHE["plan"]
    key = ("nc", bpc)
    if key not in _CACHE:
        _CACHE[key] = _build_program(reps=1, bpc=bpc)
    nc = _CACHE[key]

    res = run_bass_kernel_spmd(nc, maps, core_ids=list(range(N_CORES)),
                               **_CACHE.get("run_kwargs", {}))
    _CACHE["last_result"] = res
    return _unpack(res, inputs)


# revision 12
# speedup vs baseline: 1.1205x; 1.1205x over previous
"""Trainium2 Bass kernel for nn_EquivariantProductBasisBlock.

Computation (per node n, channel c):
  s = nf[n,c,0]; v = nf[n,c,1:4]; v2 = |v|^2
  out0 = w0*s + w1*s^2 + w2'*v2 + w3*s^3 + w4*s*v2      (w_p = W0[sp[n],p,c])
  B1   = u0 + u1'*s + u2'*s^2 + u3'*v2                  (u_p = W1[sp[n],p,c])
  out1m = B1 * v_m
  y0 = out0 @ L0 / sqrt(C);  y1m = out1m @ L1 / sqrt(C)
  y[n,c,:] = [y0, y1x, y1y, y1z] + sc[n,c,:]

Design (vs the node-major baseline):
  - Host sorts nodes by species and pads each species segment to a
    512-node block, so every device block is single-species.  The
    per-element path weights then become per-(block, channel) constants,
    delivered as a tiny [128, blocks*9] f32 table: the gather matmuls,
    their PSUM evacuation, and all PE transposes disappear.
  - Channels live on the partition axis ("cT layout"); nodes stream on
    the free axis.  Host pre-transposes nf/sc to bf16 component planes
    [c, 4, n], so the channel-mixing matmuls need no transposes at all:
    y0 = matmul(lhsT=L0, rhs=out0), y1m = matmul(lhsT=L1, rhs=out1m).
  - Weighted poly terms use dual-op tensor_scalar (x*w + b in one DVE op
    at 4x bf16 rate) with per-partition scalar APs from the table.
  - All IO is bf16 (host pre-casts): 24 B/node HBM traffic vs 48 f32.
  - sc is added on the idle TensorE: an identity matmul accumulates the
    sc planes into the same PSUM banks as the channel-mix matmuls (the
    SWDGE accumulate-DMA CCE path wedges this hardware for runs >2048
    elements, and engines have no spare throughput for a tensor add).
  - Output stored bf16, upcast on host.
  - Work is spread across engines: ACT does vx^2/vy^2 and the PSUM->SBUF
    evacuation, GPSIMD does two of the three B1*v planes, DVE does the
    rest of the elementwise chain, TensorE does channel mixing + sc.

Sharding: data-parallel over sorted node blocks across 8 cores
(8704 nodes/core incl. ~6% species padding).

Measured (interleaved reps=1 vs reps=24 A/B, this container): per-pass HW
time ~29-57 us across sessions (RPC-noise limited); the baseline kernel
(node-major f32, gather matmuls + PE transposes) measured 101968 ns with
the same methodology family.  Correctness: rel err 5.5e-3 (gate 2e-2),
dominated by bf16 input/output rounding.
"""

import numpy as np

N_CORES = 8
N_NODES = 65536
C = 128
E = 10
BLK = 512                      # single-species device block (1 PSUM bank / plane)
BLKS_PER_CHUNK = 4             # 2048-node DMA chunks (2 MiB bf16)

INV_SQ3 = 1.0 / np.sqrt(3.0)
SQ2 = float(np.sqrt(2.0))
SQ3 = float(np.sqrt(3.0))
SQ35 = float(np.sqrt(3.0 / 5.0))

_CACHE = {}


# ---------------------------------------------------------------------------
# Workarounds for the walrus build in this container: it rejects any
# instruction carrying more than one sync-wait ("Too many sync wait
# commands").  Split extra waits onto same-engine NOPs preceding the
# instruction (identical semantics: the engine queue is FIFO).
# ---------------------------------------------------------------------------
def _apply_patches():
    import concourse.tile as tile
    from concourse import mybir
    from concourse.vector_clock import ScopedClock

    if getattr(tile.TileContext, "_singlewait_patched", False):
        return

    def _patched_drain_and_barrier(self, tick_clock, wait_clock):
        nc = self.nc
        probe = nc.sync.nop()
        wait_clock.add_sem_waits(probe.ins, ScopedClock({None: tick_clock.global_clock}))
        si = probe.ins.sync_info
        waits = list(si.on_wait) if si and si.on_wait else []
        if len(waits) > 1:
            probe.ins.sync_info = type(si)(on_wait=waits[:1], on_update=[])
            for w in waits[1:]:
                extra = nc.sync.nop()
                extra.ins.sync_info = type(si)(on_wait=[w], on_update=[])
        nc.sync.drain()
        nc.all_engine_barrier()
        assert self.sems is not None
        popped = nc._tile_sem_poison_stack.pop()
        assert popped is self._sem_poison
        nc.clear_and_free_semaphores(list(self.sems.allocated().values()))
        nc.all_engine_barrier()

    _orig_commit = tile.TileContext._commit_instruction

    def _split_commit(self, inst, lazy_reg_writes=True):
        si = getattr(inst, "sync_info", None)
        if (si is not None and si.on_wait and len(si.on_wait) > 1
                and getattr(inst, "engine", mybir.EngineType.Unassigned)
                != mybir.EngineType.Unassigned):
            waits = list(si.on_wait)
            for w in waits[:-1]:
                nop = mybir.InstNoOp(name=self.nc.get_next_instruction_name(),
                                     ins=[], outs=[], engine=inst.engine)
                nop.sync_info = mybir.SyncInfo(on_wait=[w], on_update=[])
                _orig_commit(self, nop, lazy_reg_writes=False)
            inst.sync_info = mybir.SyncInfo(on_wait=[waits[-1]],
                                            on_update=list(si.on_update or []))
        return _orig_commit(self, inst, lazy_reg_writes)

    tile.TileContext._drain_and_barrier = _patched_drain_and_barrier
    tile.TileContext._commit_instruction = _split_commit
    tile.TileContext._singlewait_patched = True


def _chunk_sizes(bpc):
    """Chunk list (in blocks) for one core: full chunks + one tail."""
    sizes = [BLKS_PER_CHUNK] * (bpc // BLKS_PER_CHUNK)
    if bpc % BLKS_PER_CHUNK:
        sizes.append(bpc % BLKS_PER_CHUNK)
    return sizes


def _build_program(reps=1, bpc=17):
    import concourse.bass as bass
    import concourse.tile as tile
    from concourse import mybir
    from concourse.masks import make_identity
    from contextlib import ExitStack

    _apply_patches()
    F32 = mybir.dt.float32
    BF16 = mybir.dt.bfloat16
    nc = bass.Bass()

    P = bpc * BLK                      # nodes per core
    sizes = _chunk_sizes(bpc)

    nf_d = nc.declare_dram_parameter("nf", [C, 4 * P], BF16, isOutput=False)
    sc_d = nc.declare_dram_parameter("sc", [C, 4 * P], BF16, isOutput=False)
    w_d = nc.declare_dram_parameter("wt", [C, bpc * 9], F32, isOutput=False)
    l0_d = nc.declare_dram_parameter("l0", [C, C], BF16, isOutput=False)
    l1_d = nc.declare_dram_parameter("l1", [C, C], BF16, isOutput=False)
    out_d = nc.declare_dram_parameter("out", [C, 4 * P], BF16, isOutput=True)

    mult = mybir.AluOpType.mult
    add = mybir.AluOpType.add
    Square = mybir.ActivationFunctionType.Square

    with tile.TileContext(nc) as tc, ExitStack() as ctx:
        consts = ctx.enter_context(tc.tile_pool(name="consts", bufs=1))
        chunks = ctx.enter_context(tc.tile_pool(name="chunks", bufs=2))
        work = ctx.enter_context(tc.tile_pool(name="work", bufs=2))
        psY = ctx.enter_context(tc.tile_pool(name="psY", bufs=2, space="PSUM"))

        t_w = consts.tile([C, bpc * 9], F32)
        nc.sync.dma_start(out=t_w, in_=w_d[:, :])
        t_l0 = consts.tile([C, C], BF16)
        nc.sync.dma_start(out=t_l0, in_=l0_d[:, :])
        t_l1 = consts.tile([C, C], BF16)
        nc.sync.dma_start(out=t_l1, in_=l1_d[:, :])
        ident = consts.tile([C, C], BF16)
        make_identity(nc, ident)

        def ap(t, off, *dims):
            return bass.AP(tensor=t.tensor, offset=t.offset + off,
                           ap=[t.ap[0], *list(dims)])

        # iterate (rep, chunk)
        sched = []
        for _ in range(reps):
            base = 0
            for nb in sizes:
                sched.append((base, nb))
                base += nb
        for (cblk, nb) in sched:
            CH = nb * BLK
            col0 = 4 * cblk * BLK          # column offset of this chunk

            t_nf = chunks.tile([C, 4 * CH], BF16, tag="nf")
            nc.sync.dma_start(out=t_nf, in_=nf_d[:, col0:col0 + 4 * CH])
            t_sc = chunks.tile([C, 4 * CH], BF16, tag="sc")
            # sc load + y store ride the qAct HWDGE ring so they never queue
            # behind the next chunk's nf load on qSP (FIFO per issuing engine)
            nc.scalar.dma_start(out=t_sc, in_=sc_d[:, col0:col0 + 4 * CH])
            t_y = chunks.tile([C, 4 * CH], BF16, tag="y")

            t_vsq = work.tile([C, 2 * CH], BF16, tag="vsq")
            t_v2 = work.tile([C, CH], BF16, tag="v2")
            t_hb = work.tile([C, 2 * CH], BF16, tag="hb")    # [h|b]
            t_g1 = work.tile([C, CH], BF16, tag="g1")
            t_r = work.tile([C, CH], BF16, tag="r")
            t_X = work.tile([C, 4 * CH], BF16, tag="x")      # [out0|o1x|o1y|o1z]

            S = t_nf[:, 0:CH]
            vz = t_nf[:, 3 * CH:4 * CH]

            # |v|^2: ACT squares vx,vy; DVE squares vz and sums
            nc.scalar.activation(out=t_vsq, in_=t_nf[:, CH:3 * CH], func=Square)
            nc.vector.tensor_tensor(out=t_v2, in0=vz, in1=vz, op=mult)
            nc.vector.tensor_tensor(out=t_v2, in0=t_v2,
                                    in1=t_vsq[:, 0:CH], op=add)
            nc.vector.tensor_tensor(out=t_v2, in0=t_v2,
                                    in1=t_vsq[:, CH:2 * CH], op=add)

            def wcol(b, j):
                k = (cblk + b) * 9 + j
                return t_w[:, k:k + 1]

            # per-block weighted affines (dual-op tensor_scalar, 4x bf16)
            for b in range(nb):
                sb = S[:, b * BLK:(b + 1) * BLK]
                v2b = t_v2[:, b * BLK:(b + 1) * BLK]
                nc.vector.tensor_scalar(out=t_hb[:, b * BLK:(b + 1) * BLK],
                                        in0=sb, scalar1=wcol(b, 3),
                                        scalar2=wcol(b, 1), op0=mult, op1=add)
                nc.vector.tensor_scalar(out=t_hb[:, CH + b * BLK:CH + (b + 1) * BLK],
                                        in0=sb, scalar1=wcol(b, 7),
                                        scalar2=wcol(b, 6), op0=mult, op1=add)
                nc.vector.tensor_scalar(out=t_g1[:, b * BLK:(b + 1) * BLK],
                                        in0=sb, scalar1=wcol(b, 4),
                                        scalar2=wcol(b, 2), op0=mult, op1=add)
                nc.vector.tensor_scalar(out=t_r[:, b * BLK:(b + 1) * BLK],
                                        in0=v2b, scalar1=wcol(b, 8),
                                        scalar2=wcol(b, 5), op0=mult, op1=add)

            # [h2|b2] = [h1|b1] * s  (s broadcast over both halves)
            nc.vector.tensor_tensor(out=t_hb, in0=t_hb,
                                    in1=ap(t_nf, 0, [0, 2], [1, CH]), op=mult)

            # h3 = h2 + w0 (per block)
            for b in range(nb):
                nc.vector.tensor_scalar(out=t_hb[:, b * BLK:(b + 1) * BLK],
                                        in0=t_hb[:, b * BLK:(b + 1) * BLK],
                                        scalar1=wcol(b, 0), scalar2=None, op0=add)

            # out0 = h3*s + g1*v2
            nc.vector.tensor_tensor(out=t_X[:, 0:CH], in0=t_hb[:, 0:CH],
                                    in1=S, op=mult)
            nc.vector.tensor_tensor(out=t_g1, in0=t_g1, in1=t_v2, op=mult)
            nc.vector.tensor_tensor(out=t_X[:, 0:CH], in0=t_X[:, 0:CH],
                                    in1=t_g1, op=add)
            # B1 = b2 + r
            nc.vector.tensor_tensor(out=t_hb[:, CH:2 * CH],
                                    in0=t_hb[:, CH:2 * CH], in1=t_r, op=add)
            # out1 = B1 * v   (x,y on GpSimd, z on DVE)
            nc.gpsimd.tensor_tensor(out=t_X[:, CH:3 * CH],
                                    in0=ap(t_hb, CH, [0, 2], [1, CH]),
                                    in1=t_nf[:, CH:3 * CH], op=mult)
            nc.vector.tensor_tensor(out=t_X[:, 3 * CH:4 * CH],
                                    in0=t_hb[:, CH:2 * CH], in1=vz, op=mult)

            # channel mixing + sc (identity-matmul accumulate) + evacuation
            for b in range(nb):
                t_py = psY.tile([C, 4 * BLK], F32, tag="py")
                for m in range(4):
                    nc.tensor.matmul(t_py[:, m * BLK:(m + 1) * BLK],
                                     lhsT=(t_l0 if m == 0 else t_l1),
                                     rhs=t_X[:, m * CH + b * BLK:
                                             m * CH + (b + 1) * BLK],
                                     start=True, stop=False)
                    nc.tensor.matmul(t_py[:, m * BLK:(m + 1) * BLK],
                                     lhsT=ident,
                                     rhs=t_sc[:, m * CH + b * BLK:
                                              m * CH + (b + 1) * BLK],
                                     start=False, stop=True)
                # PSUM f32 -> y chunk bf16, interleaving planes into [4, CH]
                nc.scalar.copy(out=ap(t_y, b * BLK, [CH, 4], [1, BLK]),
                               in_=t_py[:, :])

            nc.scalar.dma_start(out=out_d[:, col0:col0 + 4 * CH], in_=t_y)

    return nc


# ---------------------------------------------------------------------------
# Host-side data plan: sort nodes by species, pad each species segment to a
# 512 multiple, split into 8 equal per-core ranges of whole blocks.
# ---------------------------------------------------------------------------
def _plan(species):
    order = np.argsort(species, kind="stable")
    counts = np.bincount(species, minlength=E)
    seg_padded = ((counts + BLK - 1) // BLK) * BLK
    n_blocks_real = int(seg_padded.sum()) // BLK
    bpc = -(-n_blocks_real // N_CORES)
    P = bpc * BLK
    L = N_CORES * P
    idx = np.full(L, -1, np.int64)
    blk_sp = np.zeros(L // BLK, np.int64)
    pos = 0
    off = 0
    for e in range(E):
        c = int(counts[e])
        idx[pos:pos + c] = order[off:off + c]
        blk_sp[pos // BLK:(pos + int(seg_padded[e])) // BLK] = e
        off += c
        pos += int(seg_padded[e])
    return idx, blk_sp, bpc, P


def _prep_host(inputs):
    species = np.asarray(inputs["node_species"]).astype(np.int64)
    idx, blk_sp, bpc, P = _plan(species)
    L = N_CORES * P

    nf = np.ascontiguousarray(np.asarray(inputs["node_feats"], dtype=np.float32))
    sc = np.ascontiguousarray(np.asarray(inputs["sc"], dtype=np.float32))
    W0 = np.asarray(inputs["W0"], dtype=np.float32)
    W1 = np.asarray(inputs["W1"], dtype=np.float32)
    L0 = np.asarray(inputs["L0"], dtype=np.float32)
    L1 = np.asarray(inputs["L1"], dtype=np.float32)

    valid = idx >= 0
    nf_s = np.zeros((L, C, 4), np.float32)
    nf_s[valid] = nf[idx[valid]]
    sc_s = np.zeros((L, C, 4), np.float32)
    sc_s[valid] = sc[idx[valid]]

    # per-element path weights, CG constants folded in
    w0 = W0.copy()
    w0[:, 2, :] *= INV_SQ3
    u = W1.copy()
    u[:, 1, :] *= SQ2
    u[:, 2, :] *= SQ3
    u[:, 3, :] *= SQ35
    wall = np.concatenate([w0, u], axis=1)          # [E, 9, C]

    inv_sqrt_c = np.float32(1.0 / np.sqrt(C))
    l0 = np.ascontiguousarray(L0 * inv_sqrt_c)
    l1 = np.ascontiguousarray(L1 * inv_sqrt_c)
    return nf_s, sc_s, wall, l0, l1, idx, blk_sp, bpc, P


def _planes(arr_s, core, bpc, P, bf16):
    """[L,C,4] f32 -> [C, 4P] bf16 chunk-major component planes for one core."""
    a = arr_s[core * P:(core + 1) * P]              # [P, C, 4]
    cols = []
    pos = 0
    for nb in _chunk_sizes(bpc):
        CH = nb * BLK
        ch = a[pos:pos + CH]                        # [CH, C, 4]
        cols.append(ch.transpose(1, 2, 0).reshape(C, 4 * CH))
        pos += CH
    return np.ascontiguousarray(np.concatenate(cols, axis=1)).astype(bf16)


def _in_maps(inputs):
    import ml_dtypes
    bf16 = ml_dtypes.bfloat16
    nf_s, sc_s, wall, l0, l1, idx, blk_sp, bpc, P = _prep_host(inputs)
    _CACHE["plan"] = (idx, bpc, P)
    l0b = l0.astype(bf16)
    l1b = l1.astype(bf16)
    maps = []
    for k in range(N_CORES):
        bs = blk_sp[k * bpc:(k + 1) * bpc]
        wt = wall[bs]                               # [bpc, 9, C]
        wt = np.ascontiguousarray(wt.transpose(2, 0, 1).reshape(C, bpc * 9))
        maps.append({
            "nf": _planes(nf_s, k, bpc, P, bf16),
            "sc": _planes(sc_s, k, bpc, P, bf16),
            "wt": wt.astype(np.float32),
            "l0": l0b,
            "l1": l1b,
        })
    return maps


def _unpack(res, inputs):
    idx, bpc, P = _CACHE["plan"]
    L = N_CORES * P
    out_s = np.empty((L, C, 4), np.float32)
    for k in range(N_CORES):
        o = np.asarray(res.results[k]["out"]).astype(np.float32)   # [C, 4P]
        pos = 0
        col = 0
        for nb in _chunk_sizes(bpc):
            CH = nb * BLK
            ch = o[:, col:col + 4 * CH].reshape(C, 4, CH)
            out_s[k * P + pos:k * P + pos + CH] = ch.transpose(2, 0, 1)
            pos += CH
            col += 4 * CH
    valid = idx >= 0
    y = np.empty((N_NODES, C, 4), np.float32)
    y[idx[valid]] = out_s[valid]
    return y


def kernel(**inputs):
    from concourse.bass_utils import run_bass_kernel_spmd

    maps = _in_maps(inputs)
    idx, bpc, P = _CACHE["plan"]
    key = ("nc", bpc)
    if key not in _CACHE:
        _CACHE[key] = _build_program(reps=1, bpc=bpc)
    nc = _CACHE[key]

    res = run_bass_kernel_spmd(nc, maps, core_ids=list(range(N_CORES)),
                               **_CACHE.get("run_kwargs", {}))
    _CACHE["last_result"] = res
    return _unpack(res, inputs)
